# revision 22
# baseline (speedup 1.0000x reference)
"""Trainium2 Bass kernel for nn_DiffusionModel (theta_post_prob).

Math (per batch b, with runtime scalars a = alphas-gather, ca = cumalphas-gather):
    p     = a*xt + k1                 k1 = (1-a)/C
    M     = ca*I + u*ones             u  = (1-ca)/C   (C x C, symmetric, stochastic)
    denom = M^T p = a*(M^T xt) + k1   (column sums of M are 1)
    g     = theta_x0 / denom
    out   = p * (M g)

Kernel layout: batch b -> core b (pure data parallel, 8 cores). Per core the
(C=32, HW=65536) slab is processed as [128, N] tiles where the 128 partitions
pack G=4 independent spatial blocks x 32 classes. Both class-reductions
(+ their broadcasts + the diagonal term) are single PE matmuls against
block-diagonal 128x128 matrices kron(a*M, I4) / kron(M, I4) built on host
(partition p = class*4 + block, so DRAM rows sit at a uniform 64 KiB stride).
"""

import os
import sys

if "/opt/trn_rl_repo" not in sys.path:
    sys.path.insert(0, "/opt/trn_rl_repo")

import numpy as np

import concourse.bacc as bacc
import concourse.mybir as mybir
from concourse.tile import TileContext
from concourse.bass_utils import run_bass_kernel_spmd

F32 = mybir.dt.float32

T = 1000
C = 32
B = 8
H = 256
W = 256
HW = H * W

NCORES = 8
G = 4                 # spatial blocks packed into the 128 partitions
P = G * C             # 128
COLS = HW // G        # 16384 columns per spatial block
MM_N = 512            # max moving free-dim per fp32 matmul


def _cfg():
    return {
        "nt": int(os.environ.get("KCFG_NT", "512")),      # compute chunk
        "ntl": int(os.environ.get("KCFG_NTL", "2048")),   # DMA tile
        "div": os.environ.get("KCFG_DIV", "recip"),       # lnexp | recip
        "mm1": os.environ.get("KCFG_MM1", "f32"),        # f32 | f32r (1st matmul)
        "mm2": os.environ.get("KCFG_MM2", "f32"),        # f32 | f32r (2nd matmul)
        "tt": os.environ.get("KCFG_TT", "gpsimd"),        # vector | gpsimd (g-mul)
        "pcomp": os.environ.get("KCFG_PCOMP", "act"),     # act | dve (p=a*x+k1 engine)
        "ysrc": os.environ.get("KCFG_YSRC", "sp"),       # sp | act (y-load HWDGE ring)
        "store": os.environ.get("KCFG_STORE", "pool"),    # pool | sp | act
        "ldbufs": int(os.environ.get("KCFG_LDBUFS", "4")),
        "wkbufs": int(os.environ.get("KCFG_WKBUFS", "4")),
        "psbufs": int(os.environ.get("KCFG_PSBUFS", "4")),
        "nstores": int(os.environ.get("KCFG_NSTORES", "1")),
        "sched": os.environ.get("KCFG_SCHED", "uniform"),      # uniform | var
    }


_CACHE = {}


def _build():
    cfg = _cfg()
    key = tuple(sorted(cfg.items()))
    if key in _CACHE:
        return _CACHE[key]

    NT = cfg["nt"]
    NTL = cfg["ntl"]
    assert NTL % NT == 0 and NT <= MM_N
    if cfg["sched"] == "var":
        # taper both ends: quick pipeline fill at the start, quick drain at the end
        widths = [1024, 1024] + [NTL] * ((COLS - 4096) // NTL) + [1024, 512, 512]
    else:
        widths = [NTL] * (COLS // NTL)
    assert sum(widths) == COLS

    nc = bacc.Bacc(
        "TRN2",
        target_bir_lowering=False,
        debug=False,
        enable_asserts=False,
        num_devices=NCORES,
    )

    xt_d = nc.dram_tensor("xt", [P, COLS], F32, kind="ExternalInput")
    x0_d = nc.dram_tensor("x0", [P, COLS], F32, kind="ExternalInput")
    ma_d = nc.dram_tensor("ma", [P, P], F32, kind="ExternalInput")
    mb_d = nc.dram_tensor("mb", [P, P], F32, kind="ExternalInput")
    sc_d = nc.dram_tensor("sc", [P, 2], F32, kind="ExternalInput")
    out_d = nc.dram_tensor("out", [P, COLS], F32, kind="ExternalOutput")

    AF = mybir.ActivationFunctionType
    store_eng = {"pool": nc.gpsimd, "sp": nc.sync, "act": nc.scalar}[cfg["store"]]

    with TileContext(nc) as tc:
        with (
            tc.tile_pool(name="consts", bufs=1) as cpool,
            tc.tile_pool(name="work", bufs=cfg["wkbufs"]) as pool,
            tc.tile_pool(name="psum", bufs=cfg["psbufs"], space="PSUM") as psum,
        ):
            ma = cpool.tile([P, P], F32)
            nc.sync.dma_start(ma[:, :], ma_d[:, :])
            mb = cpool.tile([P, P], F32)
            nc.sync.dma_start(mb[:, :], mb_d[:, :])
            sc = cpool.tile([P, 2], F32)
            nc.sync.dma_start(sc[:, :], sc_d[:, :])
            a_col = sc[:, 0:1]
            k1_col = sc[:, 1:2]

            F32R = mybir.dt.float32r
            mm1_f32r = cfg["mm1"] == "f32r"
            mm2_f32r = cfg["mm2"] == "f32r"
            if mm1_f32r:
                # SWDGE cast-DMA performs the fp32 -> f32r rounding on load
                mar = cpool.tile([P, P], F32R)
                nc.gpsimd.dma_start(mar[:, :], ma_d[:, :])
                ma_mm = mar[:, :]
            else:
                ma_mm = ma[:, :]
            if mm2_f32r:
                mbr = cpool.tile([P, P], F32R)
                nc.gpsimd.dma_start(mbr[:, :], mb_d[:, :])
                mb_mm = mbr[:, :]
            else:
                mb_mm = mb[:, :]
            tt_eng = nc.vector if cfg["tt"] == "vector" else nc.gpsimd

            off = 0
            for i, W in enumerate(widths):
                NCH = W // NT
                sl = slice(off, off + W)
                if mm1_f32r:
                    x = pool.tile([P, W], F32R, bufs=cfg["ldbufs"], tag="x",
                                  padded_shape=[P, NTL], name=f"x_{i}")
                    nc.gpsimd.dma_start(x[:, :], xt_d[:, sl])
                    x_f32 = x[:, :].bitcast(F32)
                else:
                    x = pool.tile([P, W], F32, bufs=cfg["ldbufs"], tag="x",
                                  padded_shape=[P, NTL], name=f"x_{i}")
                    nc.sync.dma_start(x[:, :], xt_d[:, sl])
                    x_f32 = x[:, :]
                y = pool.tile([P, W], F32, bufs=cfg["ldbufs"], tag="y",
                              padded_shape=[P, NTL], name=f"y_{i}")
                y_eng = nc.scalar if cfg["ysrc"] == "act" else nc.sync
                y_eng.dma_start(y[:, :], x0_d[:, sl])
                o = pool.tile([P, W], F32, bufs=cfg["ldbufs"], tag="o",
                              padded_shape=[P, NTL], name=f"o_{i}")

                dns, rdens, gs, rs = [], [], [], []
                # dn = kron(a*M, I4)^T @ x   (per group: a * M^T x)
                for j in range(0, W, NT):
                    dn = psum.tile([P, NT], F32, tag="dn", name=f"dn_{i}_{j}")
                    nc.tensor.matmul(dn[:, :], ma_mm, x[:, j:j + NT], start=True, stop=True)
                    dns.append(dn)

                # rden = 1 / (dn + k1)
                for c in range(NCH):
                    rden = pool.tile([P, NT], F32, tag="rden", name=f"rden_{i}_{c}")
                    den = pool.tile([P, NT], F32, tag="den", name=f"den_{i}_{c}")
                    nc.scalar.activation(den[:, :], dns[c][:, :], AF.Identity, bias=k1_col, scale=1.0)
                    nc.vector.reciprocal_approx_fast(out=rden[:, :], in_=den[:, :])
                    rdens.append(rden)

                # g = x0 * rden  (written as f32r so the 2nd matmul runs 1 cyc/row)
                for c in range(NCH):
                    g = pool.tile([P, NT], F32R if mm2_f32r else F32, tag="g", name=f"g_{i}_{c}")
                    tt_eng.tensor_tensor(g[:, :], y[:, c * NT:(c + 1) * NT], rdens[c][:, :],
                                         mybir.AluOpType.mult)
                    gs.append(g)

                # r = kron(M, I4)^T @ g      (per group: M g, M symmetric)
                for c in range(NCH):
                    r = psum.tile([P, NT], F32, tag="r", name=f"r_{i}_{c}")
                    nc.tensor.matmul(r[:, :], mb_mm, gs[c][:, :], start=True, stop=True)
                    rs.append(r)

                # out = (a*x + k1) * r
                for c in range(NCH):
                    js = slice(c * NT, (c + 1) * NT)
                    if cfg["pcomp"] == "act":
                        p = pool.tile([P, NT], F32, tag="p", name=f"p_{i}_{c}")
                        nc.scalar.activation(p[:, :], x_f32[:, js], AF.Identity,
                                             bias=k1_col, scale=a_col)
                        nc.vector.tensor_tensor(o[:, js], p[:, :], rs[c][:, :],
                                                mybir.AluOpType.mult)
                    else:
                        acc = pool.tile([P, 1], F32, tag="acc", name=f"acc_{i}_{c}")
                        nc.vector.affine_mul_reduce(
                            out=o[:, js], accum_out=acc[:, :], in0=x_f32[:, js],
                            in1=rs[c][:, :], scale=a_col, bias=k1_col,
                        )

                last = i == len(widths) - 1
                if last:
                    # final store: idle HWDGE ring, split halves -> shorter end drain
                    se, nstores = nc.sync, 2
                else:
                    se, nstores = store_eng, (cfg["nstores"] if W == NTL else 1)
                sw = W // nstores
                for si in range(nstores):
                    ss = slice(off + si * sw, off + (si + 1) * sw)
                    se.dma_start(out_d[:, ss], o[:, si * sw:(si + 1) * sw])
                off += W

    nc.compile()
    _CACHE[key] = nc
    return nc


def _host_prep(inputs):
    xt = np.ascontiguousarray(np.asarray(inputs["xt"], dtype=np.float32))
    x0 = np.ascontiguousarray(np.asarray(inputs["theta_x0"], dtype=np.float32))
    t = np.asarray(inputs["t"]).astype(np.int64)
    al = np.asarray(inputs["alphas"], dtype=np.float32)
    cu = np.asarray(inputs["cumalphas"], dtype=np.float32)

    eyeC = np.eye(C, dtype=np.float64)
    eyeG = np.eye(G, dtype=np.float64)
    in_maps = []
    for b in range(B):
        tm = int(t[b]) - 1
        a = 0.0 if tm == 0 else float(al[tm])
        ca = 1.0 if tm == 0 else float(cu[tm - 1])
        u = (1.0 - ca) / C
        k1 = (1.0 - a) / C
        M = ca * eyeC + u
        ma = np.kron(a * M, eyeG).astype(np.float32)
        mb = np.kron(M, eyeG).astype(np.float32)
        sc = np.empty((P, 2), dtype=np.float32)
        sc[:, 0] = a
        sc[:, 1] = k1
        in_maps.append(
            {
                "xt": xt[b].reshape(P, COLS),
                "x0": x0[b].reshape(P, COLS),
                "ma": ma,
                "mb": mb,
                "sc": sc,
            }
        )
    return in_maps


def _run(inputs, trace=False, **kw):
    nc = _build()
    in_maps = _host_prep(inputs)
    res = run_bass_kernel_spmd(
        nc, in_maps, core_ids=list(range(NCORES)), trace=trace, **kw
    )
    out = np.stack([r["out"].reshape(C, H, W) for r in res.results])
    return out, res


def kernel(**inputs):
    out, _ = _run(inputs, trace=False)
    return out


# revision 23
# speedup vs baseline: 1.0434x; 1.0434x over previous
"""Trainium2 Bass kernel for nn_DiffusionModel (theta_post_prob).

Math (per batch b, with runtime scalars a = alphas-gather, ca = cumalphas-gather):
    p     = a*xt + k1                 k1 = (1-a)/C
    M     = ca*I + u*ones             u  = (1-ca)/C   (C x C, symmetric, stochastic)
    denom = M^T p = a*(M^T xt) + k1   (column sums of M are 1)
    g     = theta_x0 / denom
    out   = p * (M g)

Kernel layout: batch b -> core b (pure data parallel, 8 cores). Per core the
(C=32, HW=65536) slab is processed as [128, N] tiles where the 128 partitions
pack G=4 independent spatial blocks x 32 classes. Both class-reductions
(+ their broadcasts + the diagonal term) are single PE matmuls against
block-diagonal 128x128 matrices kron(a*M, I4) / kron(M, I4) built on host
(partition p = class*4 + block, so DRAM rows sit at a uniform 64 KiB stride).
"""

import os
import sys

if "/opt/trn_rl_repo" not in sys.path:
    sys.path.insert(0, "/opt/trn_rl_repo")

import numpy as np

import concourse.bacc as bacc
import concourse.mybir as mybir
from concourse.tile import TileContext
from concourse.bass_utils import run_bass_kernel_spmd

F32 = mybir.dt.float32

T = 1000
C = 32
B = 8
H = 256
W = 256
HW = H * W

NCORES = 8
G = 4                 # spatial blocks packed into the 128 partitions
P = G * C             # 128
COLS = HW // G        # 16384 columns per spatial block
MM_N = 512            # max moving free-dim per fp32 matmul


def _cfg():
    return {
        "nt": int(os.environ.get("KCFG_NT", "512")),      # compute chunk
        "ntl": int(os.environ.get("KCFG_NTL", "2048")),   # DMA tile
        "div": os.environ.get("KCFG_DIV", "recip"),       # lnexp | recip
        "mm1": os.environ.get("KCFG_MM1", "f32"),        # f32 | f32r (1st matmul)
        "mm2": os.environ.get("KCFG_MM2", "f32"),        # f32 | f32r (2nd matmul)
        "tt": os.environ.get("KCFG_TT", "gpsimd"),        # vector | gpsimd (g-mul)
        "pcomp": os.environ.get("KCFG_PCOMP", "act"),     # act | dve (p=a*x+k1 engine)
        "ysrc": os.environ.get("KCFG_YSRC", "sp"),       # sp | act (y-load HWDGE ring)
        "store": os.environ.get("KCFG_STORE", "pool"),    # pool | sp | act
        "ldbufs": int(os.environ.get("KCFG_LDBUFS", "4")),
        "wkbufs": int(os.environ.get("KCFG_WKBUFS", "4")),
        "psbufs": int(os.environ.get("KCFG_PSBUFS", "4")),
        "nstores": int(os.environ.get("KCFG_NSTORES", "1")),
        "sched": os.environ.get("KCFG_SCHED", "uniform"),      # uniform | var
    }


_CACHE = {}


def _build():
    cfg = _cfg()
    key = tuple(sorted(cfg.items()))
    if key in _CACHE:
        return _CACHE[key]

    NT = cfg["nt"]
    NTL = cfg["ntl"]
    assert NTL % NT == 0 and NT <= MM_N
    if cfg["sched"] == "var":
        # taper both ends: quick pipeline fill at the start, quick drain at the end
        widths = [1024, 1024] + [NTL] * ((COLS - 4096) // NTL) + [1024, 512, 512]
    else:
        widths = [NTL] * (COLS // NTL)
    assert sum(widths) == COLS

    nc = bacc.Bacc(
        "TRN2",
        target_bir_lowering=False,
        debug=False,
        enable_asserts=False,
        num_devices=NCORES,
    )

    xt_d = nc.dram_tensor("xt", [P, COLS], F32, kind="ExternalInput")
    x0_d = nc.dram_tensor("x0", [P, COLS], F32, kind="ExternalInput")
    ma_d = nc.dram_tensor("ma", [P, P], F32, kind="ExternalInput")
    mb_d = nc.dram_tensor("mb", [P, P], F32, kind="ExternalInput")
    sc_d = nc.dram_tensor("sc", [P, 2], F32, kind="ExternalInput")
    out_d = nc.dram_tensor("out", [P, COLS], F32, kind="ExternalOutput")

    AF = mybir.ActivationFunctionType
    store_eng = {"pool": nc.gpsimd, "sp": nc.sync, "act": nc.scalar}[cfg["store"]]

    with TileContext(nc) as tc:
        with (
            tc.tile_pool(name="consts", bufs=1) as cpool,
            tc.tile_pool(name="work", bufs=cfg["wkbufs"]) as pool,
            tc.tile_pool(name="psum", bufs=cfg["psbufs"], space="PSUM") as psum,
        ):
            ma = cpool.tile([P, P], F32)
            nc.sync.dma_start(ma[:, :], ma_d[:, :])
            mb = cpool.tile([P, P], F32)
            nc.sync.dma_start(mb[:, :], mb_d[:, :])
            sc = cpool.tile([P, 2], F32)
            nc.sync.dma_start(sc[:, :], sc_d[:, :])
            a_col = sc[:, 0:1]
            k1_col = sc[:, 1:2]

            F32R = mybir.dt.float32r
            mm1_f32r = cfg["mm1"] == "f32r"
            mm2_f32r = cfg["mm2"] == "f32r"
            if mm1_f32r:
                # SWDGE cast-DMA performs the fp32 -> f32r rounding on load
                mar = cpool.tile([P, P], F32R)
                nc.gpsimd.dma_start(mar[:, :], ma_d[:, :])
                ma_mm = mar[:, :]
            else:
                ma_mm = ma[:, :]
            if mm2_f32r:
                mbr = cpool.tile([P, P], F32R)
                nc.gpsimd.dma_start(mbr[:, :], mb_d[:, :])
                mb_mm = mbr[:, :]
            else:
                mb_mm = mb[:, :]
            tt_eng = nc.vector if cfg["tt"] == "vector" else nc.gpsimd

            off = 0
            for i, W in enumerate(widths):
                NCH = W // NT
                sl = slice(off, off + W)
                if mm1_f32r:
                    x = pool.tile([P, W], F32R, bufs=cfg["ldbufs"], tag="x",
                                  padded_shape=[P, NTL], name=f"x_{i}")
                    nc.gpsimd.dma_start(x[:, :], xt_d[:, sl])
                    x_f32 = x[:, :].bitcast(F32)
                else:
                    x = pool.tile([P, W], F32, bufs=cfg["ldbufs"], tag="x",
                                  padded_shape=[P, NTL], name=f"x_{i}")
                    nc.sync.dma_start(x[:, :], xt_d[:, sl])
                    x_f32 = x[:, :]
                y = pool.tile([P, W], F32, bufs=cfg["ldbufs"], tag="y",
                              padded_shape=[P, NTL], name=f"y_{i}")
                y_eng = nc.scalar if cfg["ysrc"] == "act" else nc.sync
                y_eng.dma_start(y[:, :], x0_d[:, sl])
                o = pool.tile([P, W], F32, bufs=cfg["ldbufs"], tag="o",
                              padded_shape=[P, NTL], name=f"o_{i}")

                dns, rdens, gs, rs = [], [], [], []
                # dn = kron(a*M, I4)^T @ x   (per group: a * M^T x)
                for j in range(0, W, NT):
                    dn = psum.tile([P, NT], F32, tag="dn", name=f"dn_{i}_{j}")
                    nc.tensor.matmul(dn[:, :], ma_mm, x[:, j:j + NT], start=True, stop=True)
                    dns.append(dn)

                # rden = 1 / (dn + k1)
                for c in range(NCH):
                    rden = pool.tile([P, NT], F32, tag="rden", name=f"rden_{i}_{c}")
                    den = pool.tile([P, NT], F32, tag="den", name=f"den_{i}_{c}")
                    nc.scalar.activation(den[:, :], dns[c][:, :], AF.Identity, bias=k1_col, scale=1.0)
                    nc.vector.reciprocal_approx_fast(out=rden[:, :], in_=den[:, :])
                    rdens.append(rden)

                # g = x0 * rden  (written as f32r so the 2nd matmul runs 1 cyc/row)
                for c in range(NCH):
                    g = pool.tile([P, NT], F32R if mm2_f32r else F32, tag="g", name=f"g_{i}_{c}")
                    tt_eng.tensor_tensor(g[:, :], y[:, c * NT:(c + 1) * NT], rdens[c][:, :],
                                         mybir.AluOpType.mult)
                    gs.append(g)

                # r = kron(M, I4)^T @ g      (per group: M g, M symmetric)
                for c in range(NCH):
                    r = psum.tile([P, NT], F32, tag="r", name=f"r_{i}_{c}")
                    nc.tensor.matmul(r[:, :], mb_mm, gs[c][:, :], start=True, stop=True)
                    rs.append(r)

                # out = (a*x + k1) * r
                for c in range(NCH):
                    js = slice(c * NT, (c + 1) * NT)
                    if cfg["pcomp"] == "act":
                        p = pool.tile([P, NT], F32, tag="p", name=f"p_{i}_{c}")
                        nc.scalar.activation(p[:, :], x_f32[:, js], AF.Identity,
                                             bias=k1_col, scale=a_col)
                        nc.vector.tensor_tensor(o[:, js], p[:, :], rs[c][:, :],
                                                mybir.AluOpType.mult)
                    else:
                        acc = pool.tile([P, 1], F32, tag="acc", name=f"acc_{i}_{c}")
                        nc.vector.affine_mul_reduce(
                            out=o[:, js], accum_out=acc[:, :], in0=x_f32[:, js],
                            in1=rs[c][:, :], scale=a_col, bias=k1_col,
                        )

                nstores = cfg["nstores"] if W == NTL else 1
                sw = W // nstores
                for si in range(nstores):
                    ss = slice(off + si * sw, off + (si + 1) * sw)
                    store_eng.dma_start(out_d[:, ss], o[:, si * sw:(si + 1) * sw])
                off += W

    nc.compile()
    _CACHE[key] = nc
    return nc


def _host_prep(inputs):
    xt = np.ascontiguousarray(np.asarray(inputs["xt"], dtype=np.float32))
    x0 = np.ascontiguousarray(np.asarray(inputs["theta_x0"], dtype=np.float32))
    t = np.asarray(inputs["t"]).astype(np.int64)
    al = np.asarray(inputs["alphas"], dtype=np.float32)
    cu = np.asarray(inputs["cumalphas"], dtype=np.float32)

    eyeC = np.eye(C, dtype=np.float64)
    eyeG = np.eye(G, dtype=np.float64)
    in_maps = []
    for b in range(B):
        tm = int(t[b]) - 1
        a = 0.0 if tm == 0 else float(al[tm])
        ca = 1.0 if tm == 0 else float(cu[tm - 1])
        u = (1.0 - ca) / C
        k1 = (1.0 - a) / C
        M = ca * eyeC + u
        ma = np.kron(a * M, eyeG).astype(np.float32)
        mb = np.kron(M, eyeG).astype(np.float32)
        sc = np.empty((P, 2), dtype=np.float32)
        sc[:, 0] = a
        sc[:, 1] = k1
        in_maps.append(
            {
                "xt": xt[b].reshape(P, COLS),
                "x0": x0[b].reshape(P, COLS),
                "ma": ma,
                "mb": mb,
                "sc": sc,
            }
        )
    return in_maps


def _run(inputs, trace=False, **kw):
    nc = _build()
    in_maps = _host_prep(inputs)
    res = run_bass_kernel_spmd(
        nc, in_maps, core_ids=list(range(NCORES)), trace=trace, **kw
    )
    out = np.stack([r["out"].reshape(C, H, W) for r in res.results])
    return out, res


def kernel(**inputs):
    out, _ = _run(inputs, trace=False)
    return out
